# revision 8
# baseline (speedup 1.0000x reference)
"""Multi-head causal attention (B=2, S=2048, E=1024, H=16, D=64) on 8 TRN2 cores.

Sharding: core c -> batch b = c // 4, head group g = c % 4 (4 heads each).
Each core computes q/k/v projections + RoPE + causal attention + its rows of
the Wo projection for its (batch, head-group); the host sums the 4 row-parallel
Wo partials per batch (the unshard step of row-parallel output projection).

Device layout notes:
  - x is passed pre-transposed per batch: xT [E, S] so the PE can contract
    over E (partition dim) for the projections.
  - q/k are computed transposed (qT/kT [64, S]) with head-pair fused weights
    so one [128, 512] PSUM tile holds [q_x1; q_x2; k_x1; k_x2] rows, where
    x1/x2 are the RoPE even/odd pair halves (weight columns pre-permuted on
    host so rotate-half applies).
  - scores are computed transposed, sT [k, q] = kT.T @ qT; softmax runs over
    the partition dim via an appended ones-column in the AV matmul (Z row).
    No max-subtraction: scores ~ N(0,1), exp is safe in fp32.
  - AV computes attnT [d, q]; Wo projection contracts head-dim chunks of
    attnT against Wo rows (fp16), accumulating out [s, e] tiles in PSUM.
"""

import sys

if "/opt/trn_rl_repo" not in sys.path:
    sys.path.insert(0, "/opt/trn_rl_repo")

import numpy as np

import concourse.bass as bass
import concourse.tile as tile
from concourse import bacc, mybir
from concourse.bass_utils import run_bass_kernel_spmd

B, S, E, H, D = 2, 2048, 1024, 16, 64
HPC = 4  # heads per core
NCORES = 8
SB = 512  # q/s block width
NSB = S // SB  # 4
KT = 128  # k tile (partition chunk of the sequence)
NKT = S // KT  # 16
ECH = E // 128  # 8 contraction chunks for the projections

f32 = mybir.dt.float32
f16 = mybir.dt.float16
bf16 = mybir.dt.bfloat16

ROPE_BASE = 10000.0


def build_nc():
    nc = bacc.Bacc(
        "TRN2", target_bir_lowering=False, debug=False, enable_asserts=False
    )

    xT_d = nc.dram_tensor("xT", [E, S], f16, kind="ExternalInput")
    wqk_d = nc.dram_tensor("wqk", [E, HPC, 128], f16, kind="ExternalInput")
    wv_d = nc.dram_tensor("wv", [E, HPC * D], f16, kind="ExternalInput")
    wo_d = nc.dram_tensor("wo", [HPC * D, E], f16, kind="ExternalInput")
    cos_d = nc.dram_tensor("cos2", [128, S], f32, kind="ExternalInput")
    sin_d = nc.dram_tensor("sin2", [128, S], f32, kind="ExternalInput")
    mask_d = nc.dram_tensor("masks", [128, 4, SB], f16, kind="ExternalInput")
    out_d = nc.dram_tensor("out", [S, E], f32, kind="ExternalOutput")

    with tile.TileContext(nc) as tc:
        with (
            tc.tile_pool(name="const", bufs=1) as constp,
            tc.tile_pool(name="qk", bufs=1) as qkp,
            tc.tile_pool(name="vb", bufs=1) as vbp,
            tc.tile_pool(name="at", bufs=1) as atp,
            tc.tile_pool(name="st", bufs=6) as stp,
            tc.tile_pool(name="tmp", bufs=3) as tmpp,
            tc.tile_pool(name="mm", bufs=4, space="PSUM") as mmp,
            tc.tile_pool(name="acc", bufs=4, space="PSUM") as accp,
        ):
            # ---- constant loads -------------------------------------------------
            xT = []
            xT_ap = xT_d.ap().rearrange("(eo p) s -> eo p s", p=128)
            for e in range(ECH):
                t = constp.tile([128, S], f16, tag=f"xT{e}", name=f"xT{e}")
                nc.sync.dma_start(out=t, in_=xT_ap[e])
                xT.append(t)

            wqk = constp.tile([128, ECH, HPC, 128], f16, tag="wqk")
            nc.sync.dma_start(
                out=wqk, in_=wqk_d.ap().rearrange("(eo p) h m -> p eo h m", p=128)
            )
            wv = constp.tile([128, ECH, HPC * D], f16, tag="wv")
            nc.sync.dma_start(
                out=wv, in_=wv_d.ap().rearrange("(eo p) m -> p eo m", p=128)
            )
            wo = constp.tile([128, 2, E], f16, tag="wo")
            nc.sync.dma_start(
                out=wo, in_=wo_d.ap().rearrange("(c p) e -> p c e", p=128)
            )
            cos2 = constp.tile([128, S], f32, tag="cos2")
            nc.sync.dma_start(out=cos2, in_=cos_d.ap())
            sin2 = constp.tile([128, S], f32, tag="sin2")
            nc.sync.dma_start(out=sin2, in_=sin_d.ap())
            masks = constp.tile([128, 4, SB], f16, tag="masks")
            nc.sync.dma_start(out=masks, in_=mask_d.ap())

            # ---- phase A: fused q|k projection + RoPE ---------------------------
            # psum rows: [q_x1(32); q_x2(32); k_x1(32); k_x2(32)] for one head.
            # qq[p] rows: qT of head 2p on partitions 0-63, head 2p+1 on 64-127
            # (kk[p] likewise) so each head's scores matmul operands share a
            # partition base.
            qq = [qkp.tile([128, S], f16, tag=f"qq{p}", name=f"qq{p}") for p in range(2)]
            kk = [qkp.tile([128, S], f16, tag=f"kk{p}", name=f"kk{p}") for p in range(2)]
            swap_src = [32, 0, 96, 64]
            for h in range(HPC):
                p, half = h // 2, (h % 2) * 64
                for sb in range(NSB):
                    cs = slice(sb * SB, (sb + 1) * SB)
                    ps = mmp.tile([128, SB], f32, tag="mm", name="ps")
                    for e in range(ECH):
                        nc.tensor.matmul(
                            out=ps,
                            lhsT=wqk[:, e, h, :],
                            rhs=xT[e][:, cs],
                            start=(e == 0),
                            stop=(e == ECH - 1),
                        )
                    t1 = tmpp.tile([128, SB], f32, tag="t1", name="t1")
                    t2 = tmpp.tile([128, SB], f32, tag="t2", name="t2")
                    nc.vector.tensor_mul(t1, ps, cos2[:, cs])
                    for g in range(4):
                        srow = swap_src[g]
                        nc.vector.tensor_mul(
                            t2[g * 32 : (g + 1) * 32, :],
                            ps[srow : srow + 32, :],
                            sin2[g * 32 : (g + 1) * 32, cs],
                        )
                    nc.vector.tensor_add(
                        qq[p][half : half + 64, cs], t1[0:64, :], t2[0:64, :]
                    )
                    nc.vector.tensor_add(
                        kk[p][half : half + 64, cs], t1[64:128, :], t2[64:128, :]
                    )

            # ---- phase B: v projection (natural layout, fp16, ones column) -----
            # v_big free layout per k-chunk: 4 heads x [v_h (64) | one (1)] = 260
            v_big = vbp.tile([128, NKT, HPC * 65], f16, tag="vbig")
            ones_cols = v_big.rearrange("p n (h m) -> p n h m", h=HPC)[:, :, :, 64:65]
            nc.vector.memset(ones_cols, 1.0)
            for kc in range(NKT):
                vps = accp.tile([128, HPC * D], f32, tag="acc", name="vps")
                for e in range(ECH):
                    nc.tensor.matmul(
                        out=vps,
                        lhsT=xT[e][:, kc * KT : (kc + 1) * KT],
                        rhs=wv[:, e, :],
                        start=(e == 0),
                        stop=(e == ECH - 1),
                    )
                nc.vector.tensor_copy(
                    out=v_big.rearrange("p n (h m) -> p n h m", h=HPC)[
                        :, kc, :, 0:64
                    ],
                    in_=vps.rearrange("p (h m) -> p h m", h=HPC),
                )

            # ---- phase C: attention per (q block, head pair) --------------------
            # attnT tiles: at8[c][qb] rows = hd chunk c (2 heads x 64), cols = q
            # Heads 2p / 2p+1 sit at partition bases 0 / 64 of qq[p]/kk[p], so
            # their K=64 scores matmuls land in disjoint PE row groups and run
            # concurrently (row tiling via auto tile_position).
            at8 = {}
            for c in range(2):
                for qb in range(NSB):
                    at8[(c, qb)] = atp.tile(
                        [128, SB], f16, tag=f"at{c}_{qb}", name=f"at{c}_{qb}"
                    )

            for qb in range(NSB):
                qs = slice(qb * SB, (qb + 1) * SB)
                n_k = 4 * (qb + 1)
                for p in range(2):
                    avs = [
                        accp.tile([128, SB], f32, tag="acc", name=f"av{i}")
                        for i in range(2)
                    ]
                    for kt in range(n_k):
                        kts = slice(kt * KT, (kt + 1) * KT)
                        j = kt - 4 * qb
                        pss, sts = [], []
                        for i in range(2):
                            half = i * 64
                            ps = mmp.tile([128, SB], f32, tag="mm", name="ps")
                            nc.tensor.matmul(
                                out=ps,
                                lhsT=kk[p][half : half + 64, kts],
                                rhs=qq[p][half : half + 64, qs],
                                start=True,
                                stop=True,
                            )
                            pss.append(ps)
                        for i in range(2):
                            st_t = stp.tile([128, SB], f16, tag="st", name="st_t")
                            nc.scalar.activation(
                                out=st_t,
                                in_=pss[i],
                                func=mybir.ActivationFunctionType.Exp,
                                scale=0.125,
                            )
                            if j >= 0:
                                nc.gpsimd.tensor_mul(st_t, st_t, masks[:, j, :])
                            sts.append(st_t)
                        for i in range(2):
                            h = 2 * p + i
                            nc.tensor.matmul(
                                out=avs[i][0:65, :],
                                lhsT=v_big[:, kt, h * 65 : (h + 1) * 65],
                                rhs=sts[i],
                                start=(kt == 0),
                                stop=(kt == n_k - 1),
                            )
                    # normalize: attnT = av[0:64] / Z  (Z = av row 64)
                    for i in range(2):
                        h = 2 * p + i
                        r = tmpp.tile([1, SB], f32, tag="r", name="r")
                        nc.vector.reciprocal(out=r, in_=avs[i][64:65, :])
                        zb = tmpp.tile([64, SB], f32, tag="zb", name="zb")
                        nc.gpsimd.partition_broadcast(zb, r)
                        c, half = h // 2, (h % 2) * 64
                        nc.vector.tensor_mul(
                            at8[(c, qb)][half : half + 64, :], avs[i][0:64, :], zb
                        )

            # ---- phase D: output projection (row-parallel partial) -------------
            for qb in range(NSB):
                for stl in range(4):
                    rows = qb * SB + stl * KT
                    for eb in range(2):
                        pw = mmp.tile([128, SB], f32, tag="mm", name="pw")
                        for c in range(2):
                            nc.tensor.matmul(
                                out=pw,
                                lhsT=at8[(c, qb)][:, stl * KT : (stl + 1) * KT],
                                rhs=wo[:, c, eb * SB : (eb + 1) * SB],
                                start=(c == 0),
                                stop=(c == 1),
                            )
                        ot = stp.tile([128, SB], f32, tag="ot", name="ot", bufs=3)
                        nc.vector.tensor_copy(out=ot, in_=pw)
                        nc.sync.dma_start(
                            out=out_d.ap()[rows : rows + KT, eb * SB : (eb + 1) * SB],
                            in_=ot,
                        )

    nc.compile()
    return nc


def build_in_maps(x, Wq, Wk, Wv, Wo):
    x = np.asarray(x, np.float32)
    Wq = np.asarray(Wq, np.float32)
    Wk = np.asarray(Wk, np.float32)
    Wv = np.asarray(Wv, np.float32)
    Wo = np.asarray(Wo, np.float32)

    # RoPE tables in rotate-half layout ([32] pair-frequencies, duplicated)
    inv = 1.0 / (ROPE_BASE ** (np.arange(0, D, 2, dtype=np.float64) / D))  # [32]
    ang = inv[:, None] * np.arange(S, dtype=np.float64)[None, :]  # [32, S]
    cos_t = np.cos(ang).astype(np.float32)
    sin_t = np.sin(ang).astype(np.float32)
    cos2 = np.concatenate([cos_t, cos_t, cos_t, cos_t], 0)  # [128, S]
    sin2 = np.concatenate([-sin_t, sin_t, -sin_t, sin_t], 0)  # [128, S]

    # Causal mask tiles for the 4 diagonal offsets (keep iff q >= k)
    rr = np.arange(128)[:, None]
    cc = np.arange(SB)[None, :]
    masks = np.ascontiguousarray(
        np.stack([(cc >= rr + j * KT) for j in range(4)], axis=1)
    ).astype(np.float16)  # [128, 4, SB]

    # weight column permutation: even pair-elements then odd (rotate-half)
    perm = np.concatenate([np.arange(0, D, 2), np.arange(1, D, 2)])

    in_maps = []
    for core in range(NCORES):
        b, g = core // HPC, core % HPC
        wqk = np.empty((E, HPC, 128), np.float32)
        for i in range(HPC):
            h = g * HPC + i
            wqk[:, i, 0:64] = Wq[:, h * D : (h + 1) * D][:, perm]
            wqk[:, i, 64:128] = Wk[:, h * D : (h + 1) * D][:, perm]
        in_maps.append(
            {
                "xT": np.ascontiguousarray(x[b].T).astype(np.float16),
                "wqk": wqk.astype(np.float16),
                "wv": np.ascontiguousarray(
                    Wv[:, g * HPC * D : (g + 1) * HPC * D]
                ).astype(np.float16),
                "wo": np.ascontiguousarray(
                    Wo[g * HPC * D : (g + 1) * HPC * D, :]
                ).astype(np.float16),
                "cos2": cos2,
                "sin2": sin2,
                "masks": masks,
            }
        )
    return in_maps


def gather_output(results):
    outs = [r["out"].astype(np.float32) for r in results]
    return np.stack(
        [outs[0] + outs[1] + outs[2] + outs[3], outs[4] + outs[5] + outs[6] + outs[7]],
        axis=0,
    )


_NC_CACHE = {}


def kernel(x, Wq, Wk, Wv, Wo):
    in_maps = build_in_maps(x, Wq, Wk, Wv, Wo)
    if "nc" not in _NC_CACHE:
        _NC_CACHE["nc"] = build_nc()
    res = run_bass_kernel_spmd(_NC_CACHE["nc"], in_maps, core_ids=list(range(NCORES)))
    return gather_output(res.results)


# revision 11
# speedup vs baseline: 1.5237x; 1.5237x over previous
"""Multi-head causal attention (B=2, S=2048, E=1024, H=16, D=64) on 8 TRN2 cores.

Sharding: core c -> batch b = c // 4, head group g = c % 4 (4 heads each).
Each core computes q/k/v projections + RoPE + causal attention + its rows of
the Wo projection for its (batch, head-group); the host sums the 4 row-parallel
Wo partials per batch (the unshard step of row-parallel output projection).

Device layout notes:
  - x is passed pre-transposed per batch: xT [E, S] so the PE can contract
    over E (partition dim) for the projections.
  - q/k are computed transposed (qT/kT [64, S]) with head-pair fused weights
    so one [128, 512] PSUM tile holds [q_x1; q_x2; k_x1; k_x2] rows, where
    x1/x2 are the RoPE even/odd pair halves (weight columns pre-permuted on
    host so rotate-half applies).
  - scores are computed transposed, sT [k, q] = kT.T @ qT; softmax runs over
    the partition dim via an appended ones-column in the AV matmul (Z row).
    No max-subtraction: scores ~ N(0,1), exp is safe in fp32.
  - AV computes attnT [d, q]; Wo projection contracts head-dim chunks of
    attnT against Wo rows (fp16), accumulating out [s, e] tiles in PSUM.
"""

import sys

if "/opt/trn_rl_repo" not in sys.path:
    sys.path.insert(0, "/opt/trn_rl_repo")

import numpy as np

import concourse.bass as bass
import concourse.tile as tile
from concourse import bacc, mybir
from concourse.bass_utils import run_bass_kernel_spmd

B, S, E, H, D = 2, 2048, 1024, 16, 64
HPC = 4  # heads per core
NCORES = 8
SB = 512  # q/s block width
NSB = S // SB  # 4
KT = 128  # k tile (partition chunk of the sequence)
NKT = S // KT  # 16
ECH = E // 128  # 8 contraction chunks for the projections

f32 = mybir.dt.float32
f16 = mybir.dt.float16
bf16 = mybir.dt.bfloat16

ROPE_BASE = 10000.0


def build_nc():
    nc = bacc.Bacc(
        "TRN2", target_bir_lowering=False, debug=False, enable_asserts=False
    )

    xT_d = nc.dram_tensor("xT", [E, S], f16, kind="ExternalInput")
    wqk_d = nc.dram_tensor("wqk", [E, HPC, 128], f16, kind="ExternalInput")
    wv_d = nc.dram_tensor("wv", [E, HPC * D], f16, kind="ExternalInput")
    wo_d = nc.dram_tensor("wo", [HPC * D, E], f16, kind="ExternalInput")
    cos_d = nc.dram_tensor("cos2", [128, S], f32, kind="ExternalInput")
    sin_d = nc.dram_tensor("sin2", [128, S], f32, kind="ExternalInput")
    mask_d = nc.dram_tensor("masks", [128, 4, SB], f16, kind="ExternalInput")
    out_d = nc.dram_tensor("out", [S, E], f16, kind="ExternalOutput")

    with tile.TileContext(nc) as tc:
        with (
            tc.tile_pool(name="const", bufs=1) as constp,
            tc.tile_pool(name="qk", bufs=1) as qkp,
            tc.tile_pool(name="vb", bufs=1) as vbp,
            tc.tile_pool(name="at", bufs=1) as atp,
            tc.tile_pool(name="st", bufs=10) as stp,
            tc.tile_pool(name="tmp", bufs=4) as tmpp,
            tc.tile_pool(name="mm", bufs=5, space="PSUM") as mmp,
            tc.tile_pool(name="acc", bufs=3, space="PSUM") as accp,
        ):
            # ---- constant loads -------------------------------------------------
            xT = []
            xT_ap = xT_d.ap().rearrange("(eo p) s -> eo p s", p=128)
            for e in range(ECH):
                t = constp.tile([128, S], f16, tag=f"xT{e}", name=f"xT{e}")
                nc.sync.dma_start(out=t, in_=xT_ap[e])
                xT.append(t)

            wqk = constp.tile([128, ECH, HPC, 128], f16, tag="wqk")
            nc.sync.dma_start(
                out=wqk, in_=wqk_d.ap().rearrange("(eo p) h m -> p eo h m", p=128)
            )
            wv = constp.tile([128, ECH, HPC * D], f16, tag="wv")
            nc.sync.dma_start(
                out=wv, in_=wv_d.ap().rearrange("(eo p) m -> p eo m", p=128)
            )
            wo = constp.tile([128, 2, E], f16, tag="wo")
            nc.sync.dma_start(
                out=wo, in_=wo_d.ap().rearrange("(c p) e -> p c e", p=128)
            )
            cos2 = constp.tile([128, S], f32, tag="cos2")
            nc.sync.dma_start(out=cos2, in_=cos_d.ap())
            sin2 = constp.tile([128, S], f32, tag="sin2")
            nc.sync.dma_start(out=sin2, in_=sin_d.ap())
            masks = constp.tile([128, 4, SB], f16, tag="masks")
            nc.sync.dma_start(out=masks, in_=mask_d.ap())

            # ---- phase A: fused q|k projection + RoPE ---------------------------
            # psum rows: [q_x1(32); q_x2(32); k_x1(32); k_x2(32)] for one head.
            # qq[p] rows: qT of head 2p on partitions 0-63, head 2p+1 on 64-127
            # (kk[p] likewise) so each head's scores matmul operands share a
            # partition base.
            qq = [qkp.tile([128, S], f16, tag=f"qq{p}", name=f"qq{p}") for p in range(2)]
            kk = [qkp.tile([128, S], f16, tag=f"kk{p}", name=f"kk{p}") for p in range(2)]
            swap_src = [32, 0, 96, 64]
            for h in range(HPC):
                p, half = h // 2, (h % 2) * 64
                for sb in range(NSB):
                    cs = slice(sb * SB, (sb + 1) * SB)
                    ps = mmp.tile([128, SB], f32, tag="mm", name="ps")
                    for e in range(ECH):
                        nc.tensor.matmul(
                            out=ps,
                            lhsT=wqk[:, e, h, :],
                            rhs=xT[e][:, cs],
                            start=(e == 0),
                            stop=(e == ECH - 1),
                        )
                    t1 = tmpp.tile([128, SB], f32, tag="t1", name="t1")
                    t2 = tmpp.tile([128, SB], f32, tag="t2", name="t2")
                    nc.vector.tensor_mul(t1, ps, cos2[:, cs])
                    for g in range(4):
                        srow = swap_src[g]
                        nc.vector.tensor_mul(
                            t2[g * 32 : (g + 1) * 32, :],
                            ps[srow : srow + 32, :],
                            sin2[g * 32 : (g + 1) * 32, cs],
                        )
                    nc.vector.tensor_add(
                        qq[p][half : half + 64, cs], t1[0:64, :], t2[0:64, :]
                    )
                    nc.vector.tensor_add(
                        kk[p][half : half + 64, cs], t1[64:128, :], t2[64:128, :]
                    )

            # ---- phase B: v projection (natural layout, fp16, ones column) -----
            # v_big free layout per k-chunk: 4 heads x [v_h (64) | one (1)] = 260
            v_big = vbp.tile([128, NKT, HPC * 65], f16, tag="vbig")
            ones_cols = v_big.rearrange("p n (h m) -> p n h m", h=HPC)[:, :, :, 64:65]
            nc.vector.memset(ones_cols, 1.0)
            for kc in range(NKT):
                vps = accp.tile([128, HPC * D], f32, tag="acc", name="vps")
                for e in range(ECH):
                    nc.tensor.matmul(
                        out=vps,
                        lhsT=xT[e][:, kc * KT : (kc + 1) * KT],
                        rhs=wv[:, e, :],
                        start=(e == 0),
                        stop=(e == ECH - 1),
                    )
                nc.vector.tensor_copy(
                    out=v_big.rearrange("p n (h m) -> p n h m", h=HPC)[
                        :, kc, :, 0:64
                    ],
                    in_=vps.rearrange("p (h m) -> p h m", h=HPC),
                )

            # ---- phase C: attention per (q block, head pair) --------------------
            # attnT tiles: at8[c][qb] rows = hd chunk c (2 heads x 64), cols = q
            # Heads 2p / 2p+1 sit at partition bases 0 / 64 of qq[p]/kk[p], so
            # their K=64 scores matmuls land in disjoint PE row groups and run
            # concurrently (row tiling via auto tile_position).
            at8 = {}
            for c in range(2):
                for qb in range(NSB):
                    at8[(c, qb)] = atp.tile(
                        [128, SB], f16, tag=f"at{c}_{qb}", name=f"at{c}_{qb}"
                    )

            for qb in range(NSB):
                qs = slice(qb * SB, (qb + 1) * SB)
                n_k = 4 * (qb + 1)
                for p in range(2):
                    avs = [
                        accp.tile([128, SB], f32, tag="acc", name=f"av{i}")
                        for i in range(2)
                    ]
                    # Software pipeline: emit the AV matmul for chunk kt only
                    # LAG steps after its scores matmul, so the PE (strict
                    # in-order queue) never head-of-line blocks on the ACT exp.
                    LAG = 3
                    sts_buf = {}
                    for step in range(n_k + LAG):
                        if step < n_k:
                            kt = step
                            j = kt - 4 * qb
                            kts = slice(kt * KT, (kt + 1) * KT)
                            pss, sts = [], []
                            for i in range(2):
                                half = i * 64
                                ps = mmp.tile([128, SB], f32, tag="mm", name="ps")
                                nc.tensor.matmul(
                                    out=ps,
                                    lhsT=kk[p][half : half + 64, kts],
                                    rhs=qq[p][half : half + 64, qs],
                                    start=True,
                                    stop=True,
                                )
                                pss.append(ps)
                            for i in range(2):
                                st_t = stp.tile(
                                    [128, SB], f16, tag="st", name="st_t"
                                )
                                nc.scalar.activation(
                                    out=st_t,
                                    in_=pss[i],
                                    func=mybir.ActivationFunctionType.Exp,
                                    scale=0.125,
                                )
                                if j >= 0:
                                    nc.gpsimd.tensor_mul(
                                        st_t, st_t, masks[:, j, :]
                                    )
                                sts.append(st_t)
                            sts_buf[kt] = sts
                        if step >= LAG:
                            kt = step - LAG
                            sts = sts_buf.pop(kt)
                            for i in range(2):
                                h = 2 * p + i
                                nc.tensor.matmul(
                                    out=avs[i][0:65, :],
                                    lhsT=v_big[:, kt, h * 65 : (h + 1) * 65],
                                    rhs=sts[i],
                                    start=(kt == 0),
                                    stop=(kt == n_k - 1),
                                )
                    # normalize: attnT = av[0:64] / Z  (Z = av row 64)
                    for i in range(2):
                        h = 2 * p + i
                        r = tmpp.tile([1, SB], f32, tag="r", name="r")
                        nc.vector.reciprocal(out=r, in_=avs[i][64:65, :])
                        zb = tmpp.tile([64, SB], f32, tag="zb", name="zb")
                        nc.gpsimd.partition_broadcast(zb, r)
                        c, half = h // 2, (h % 2) * 64
                        nc.vector.tensor_mul(
                            at8[(c, qb)][half : half + 64, :], avs[i][0:64, :], zb
                        )

            # ---- phase D: output projection (row-parallel partial) -------------
            for qb in range(NSB):
                for stl in range(4):
                    rows = qb * SB + stl * KT
                    for eb in range(2):
                        pw = mmp.tile([128, SB], f32, tag="mm", name="pw")
                        for c in range(2):
                            nc.tensor.matmul(
                                out=pw,
                                lhsT=at8[(c, qb)][:, stl * KT : (stl + 1) * KT],
                                rhs=wo[:, c, eb * SB : (eb + 1) * SB],
                                start=(c == 0),
                                stop=(c == 1),
                            )
                        ot = stp.tile([128, SB], f16, tag="ot", name="ot", bufs=3)
                        nc.scalar.copy(out=ot, in_=pw)
                        nc.sync.dma_start(
                            out=out_d.ap()[rows : rows + KT, eb * SB : (eb + 1) * SB],
                            in_=ot,
                        )

    nc.compile()
    return nc


def build_in_maps(x, Wq, Wk, Wv, Wo):
    x = np.asarray(x, np.float32)
    Wq = np.asarray(Wq, np.float32)
    Wk = np.asarray(Wk, np.float32)
    Wv = np.asarray(Wv, np.float32)
    Wo = np.asarray(Wo, np.float32)

    # RoPE tables in rotate-half layout ([32] pair-frequencies, duplicated)
    inv = 1.0 / (ROPE_BASE ** (np.arange(0, D, 2, dtype=np.float64) / D))  # [32]
    ang = inv[:, None] * np.arange(S, dtype=np.float64)[None, :]  # [32, S]
    cos_t = np.cos(ang).astype(np.float32)
    sin_t = np.sin(ang).astype(np.float32)
    cos2 = np.concatenate([cos_t, cos_t, cos_t, cos_t], 0)  # [128, S]
    sin2 = np.concatenate([-sin_t, sin_t, -sin_t, sin_t], 0)  # [128, S]

    # Causal mask tiles for the 4 diagonal offsets (keep iff q >= k)
    rr = np.arange(128)[:, None]
    cc = np.arange(SB)[None, :]
    masks = np.ascontiguousarray(
        np.stack([(cc >= rr + j * KT) for j in range(4)], axis=1)
    ).astype(np.float16)  # [128, 4, SB]

    # weight column permutation: even pair-elements then odd (rotate-half)
    perm = np.concatenate([np.arange(0, D, 2), np.arange(1, D, 2)])

    in_maps = []
    for core in range(NCORES):
        b, g = core // HPC, core % HPC
        wqk = np.empty((E, HPC, 128), np.float32)
        for i in range(HPC):
            h = g * HPC + i
            wqk[:, i, 0:64] = Wq[:, h * D : (h + 1) * D][:, perm]
            wqk[:, i, 64:128] = Wk[:, h * D : (h + 1) * D][:, perm]
        in_maps.append(
            {
                "xT": np.ascontiguousarray(x[b].T).astype(np.float16),
                "wqk": wqk.astype(np.float16),
                "wv": np.ascontiguousarray(
                    Wv[:, g * HPC * D : (g + 1) * HPC * D]
                ).astype(np.float16),
                "wo": np.ascontiguousarray(
                    Wo[g * HPC * D : (g + 1) * HPC * D, :]
                ).astype(np.float16),
                "cos2": cos2,
                "sin2": sin2,
                "masks": masks,
            }
        )
    return in_maps


def gather_output(results):
    outs = [np.asarray(r["out"], np.float32) for r in results]
    return np.stack(
        [outs[0] + outs[1] + outs[2] + outs[3], outs[4] + outs[5] + outs[6] + outs[7]],
        axis=0,
    )


_NC_CACHE = {}


def kernel(x, Wq, Wk, Wv, Wo):
    in_maps = build_in_maps(x, Wq, Wk, Wv, Wo)
    if "nc" not in _NC_CACHE:
        _NC_CACHE["nc"] = build_nc()
    res = run_bass_kernel_spmd(_NC_CACHE["nc"], in_maps, core_ids=list(range(NCORES)))
    return gather_output(res.results)
